# revision 1
# baseline (speedup 1.0000x reference)
"""Trainium2 Bass kernel for nn_Encoder_18726057410744 (3-layer GIN + BatchNorm +
projection head) distributed over 8 NeuronCores.

Strategy (feature-major activations):
  - Nodes sharded by destination across 8 cores (contiguous ranges of N/8).
  - Per layer, each core gathers source-node rows for its edges from a full
    node-major z table in its local HBM via dma_gather (int16 indices; the
    table is addressed through two base slices [0:32768) and [N-32768, N) to
    cover row ids >= 32768).
  - segment_sum runs on the PE: for each 128-edge chunk belonging to one
    128-dst tile, a weighted one-hot mask M[e, d] = w_e * (dstl_e == d) is
    built on the DVE (is_equal vs an iota row, then scaled), and
    psum[f, d] += zg[e, f].T @ M accumulates the aggregation feature-major.
  - The GIN self term (1+eps)*z is folded in as per-node self-edges with
    weight 1+eps baked per layer on the host.
  - MLP: lhsT = W (natural layout) with feature-major activations; ACT fuses
    bias+ReLU out of PSUM. Second matmul uses lhsT = h1 to emit node-major z
    directly; bias added via a K=1 ones-row matmul.
  - Between layers an AllGather rebuilds the full node-major z table.
  - BatchNorm batch stats via ones-column matmuls accumulated in PSUM + a
    tiny AllReduce; normalize + projection + PReLU done per shard.

The host reorders/pads edges so every core runs an identical instruction
schedule (required: one NEFF runs SPMD on all 8 cores).
"""

import os
import sys

import numpy as np

for _p in ("/opt/trn_rl_repo",):
    if os.path.isdir(_p) and _p not in sys.path:
        sys.path.insert(0, _p)

import concourse.bacc as bacc
import concourse.bass as bass
import concourse.mybir as mybir
import concourse.tile as tile
from concourse.bass_utils import run_bass_kernel_spmd

F32 = mybir.dt.float32
I16 = mybir.dt.int16
AF = mybir.ActivationFunctionType
ALU = mybir.AluOpType

P = 128          # partitions / tile edge
A_LIM = 32768    # int16 index limit: table A covers rows [0, A_LIM)
BN_EPS = 1e-5


# ----------------------------------------------------------------------------
# Host-side preprocessing
# ----------------------------------------------------------------------------

class Sched:
    """Static (uniform-across-cores) schedule + per-core input arrays."""
    __slots__ = (
        "n_cores", "N", "NPC", "NT", "GROUP", "groups",
        "nch", "chunk_off", "call_list", "calls_by_group",
        "NCHTOT", "IDXCOLS", "NCHP_MAX",
        "idx16", "dstl", "wts",
    )


def _preprocess(edge_index, edge_weight, one_plus_eps, N, n_cores, group=3):
    """Partition edges by destination, build padded chunk metadata.

    Chunk structure is static across cores: for each (tile, part) the chunk
    count is the max over cores. Part 0 gathers from table A (src < A_LIM),
    part 1 from table B (src >= A_LIM, idx = src - (N - A_LIM)).
    """
    src = np.asarray(edge_index[0], dtype=np.int64)
    dst = np.asarray(edge_index[1], dtype=np.int64)
    w = np.asarray(edge_weight, dtype=np.float32)
    E = src.shape[0]
    assert N % n_cores == 0
    NPC = N // n_cores
    NT = -(-NPC // P)          # tiles per core
    n_layers = len(one_plus_eps)
    has_b = N > A_LIM
    B_OFF = max(N - A_LIM, 0)

    # append self edges (weight placeholder; per-layer value = 1 + eps_l)
    all_ids = np.arange(N, dtype=np.int64)
    src = np.concatenate([src, all_ids])
    dst = np.concatenate([dst, all_ids])
    w = np.concatenate([w, np.ones(N, np.float32)])
    is_self = np.zeros(E + N, bool)
    is_self[E:] = True

    core_of = dst // NPC
    loc = dst % NPC
    tile_of = loc // P
    dstl = (loc % P).astype(np.float32)
    part = (src >= A_LIM).astype(np.int64) if has_b else np.zeros_like(src)
    idxv = np.where(part == 1, src - B_OFF, src).astype(np.int16)

    key = ((core_of * NT + tile_of) * 2 + part)
    order = np.argsort(key, kind="stable")
    key_s = key[order]
    idx_s, w_s, dstl_s, self_s = idxv[order], w[order], dstl[order], is_self[order]

    nkeys = n_cores * NT * 2
    starts = np.searchsorted(key_s, np.arange(nkeys))
    ends = np.searchsorted(key_s, np.arange(nkeys) + 1)
    counts = (ends - starts).reshape(n_cores, NT, 2)

    # static chunk count per (tile, part): max over cores
    nch = -(-counts.max(axis=0) // P)        # [NT, 2]
    if not has_b:
        nch[:, 1] = 0

    # group tiles; call order per group: part A of its tiles, then part B.
    # Calls are capped at MAXCH chunks (1024 idxs) — larger single gathers
    # lose DMA parallelism; small calls round-robin over 4 SWDGE queues.
    MAXCH = 8
    groups = [list(range(g, min(g + group, NT))) for g in range(0, NT, group)]
    chunk_off = np.zeros((NT, 2), np.int64)   # column offset of each (tile, part)
    call_list = []                            # flat: (part, [(t, coff, nch_seg)])
    calls_by_group = []                       # per group: [(part, entry), ...]
    off = 0
    for tiles in groups:
        gcalls = []
        for pt in (0, 1):
            seg = [(t, int(nch[t, pt])) for t in tiles if nch[t, pt] > 0]
            if not seg:
                continue
            entry = []
            room = MAXCH
            for t, c in seg:
                chunk_off[t, pt] = off
                left = c
                while left > 0:
                    take = min(left, room)
                    entry.append((t, off, take))
                    off += take
                    left -= take
                    room -= take
                    if room == 0:
                        call_list.append((pt, entry))
                        gcalls.append((pt, entry))
                        entry = []
                        room = MAXCH
            if entry:
                call_list.append((pt, entry))
                gcalls.append((pt, entry))
        calls_by_group.append(gcalls)
    NCHTOT = off
    NCHP_MAX = int(nch.max())

    sc = Sched()
    sc.n_cores, sc.N, sc.NPC, sc.NT, sc.GROUP = n_cores, N, NPC, NT, group
    sc.groups, sc.nch, sc.chunk_off, sc.call_list = groups, nch, chunk_off, call_list
    sc.calls_by_group = calls_by_group
    sc.NCHTOT = NCHTOT
    sc.IDXCOLS = NCHTOT * P // 16
    sc.NCHP_MAX = NCHP_MAX

    # per-core arrays
    idx16 = np.zeros((n_cores, 128, sc.IDXCOLS), np.int16)
    dstl_a = np.zeros((n_cores, P, NCHTOT), np.float32)
    wts_a = np.zeros((n_cores, P, n_layers * NCHTOT), np.float32)

    for c in range(n_cores):
        flat_idx = np.zeros(NCHTOT * P, np.int16)
        flat_w = np.zeros(NCHTOT * P, np.float32)
        flat_d = np.zeros(NCHTOT * P, np.float32)
        flat_self = np.zeros(NCHTOT * P, bool)
        for t in range(NT):
            for pt in (0, 1):
                cnt = counts[c, t, pt]
                if nch[t, pt] == 0:
                    continue
                s0 = starts[(c * NT + t) * 2 + pt]
                o0 = chunk_off[t, pt] * P
                flat_idx[o0:o0 + cnt] = idx_s[s0:s0 + cnt]
                flat_w[o0:o0 + cnt] = w_s[s0:s0 + cnt]
                flat_d[o0:o0 + cnt] = dstl_s[s0:s0 + cnt]
                flat_self[o0:o0 + cnt] = self_s[s0:s0 + cnt]
        # edge j of chunk k -> partition j, column k  (dma_gather: call-local
        # row i -> partition i%128, slot i//128; calls are chunk-aligned)
        dstl_a[c] = flat_d.reshape(NCHTOT, P).T
        w2d = flat_w.reshape(NCHTOT, P).T
        self2d = flat_self.reshape(NCHTOT, P).T
        for l in range(n_layers):
            wl = np.where(self2d, np.float32(one_plus_eps[l]), w2d)
            wts_a[c, :, l * NCHTOT:(l + 1) * NCHTOT] = wl
        # idx array: per call, wrap in 16 partitions, replicate to 128
        for pt, entry in call_list:
            o0 = entry[0][1] * P
            n_idx = sum(cc for _, _, cc in entry) * P
            blk = flat_idx[o0:o0 + n_idx].reshape(n_idx // 16, 16).T  # [16, n/16]
            c0 = o0 // 16
            idx16[c, :, c0:c0 + n_idx // 16] = np.tile(blk, (8, 1))

    sc.idx16, sc.dstl, sc.wts = idx16, dstl_a, wts_a
    return sc


# ----------------------------------------------------------------------------
# Kernel build
# ----------------------------------------------------------------------------

def _build(sc: Sched, n_layers=3, debug_dump=False, ablate=0):
    # ablate: 1=agg only; 2=+mlp L0; 3=+AG L0; 4=all layers no tail; 0=full
    # 5=gathers only; 6=gathers+masks (no matmuls)
    do_mlp = ablate not in (1, 5, 6, 7)
    do_agg = ablate not in (5, 6, 7)
    do_mask = ablate not in (5, 7)
    do_gather = ablate != 7
    do_coll = ablate in (0, 3, 4)
    do_tail = ablate == 0
    layers_run = 1 if ablate in (1, 2, 3, 5, 6, 7) else n_layers
    n_cores, N, NPC, NT = sc.n_cores, sc.N, sc.NPC, sc.NT
    has_b = N > A_LIM
    B_OFF = max(N - A_LIM, 0)
    last_rows = NPC - (NT - 1) * P  # valid rows in last tile

    nc = bacc.Bacc("TRN2", target_bir_lowering=False, debug=False,
                   num_devices=n_cores, num_swdge_queues=4)

    # ---- I/O ----
    xfull = nc.dram_tensor("xfull", [N, P], F32, kind="ExternalInput")
    idx16 = nc.dram_tensor("idx16", [128, sc.IDXCOLS], I16, kind="ExternalInput")
    dstl_d = nc.dram_tensor("dstl", [P, sc.NCHTOT], F32, kind="ExternalInput")
    wts_d = nc.dram_tensor("wts", [P, n_layers * sc.NCHTOT], F32, kind="ExternalInput")
    iota_d = nc.dram_tensor("iota_rep", [P, sc.NCHP_MAX * P], F32, kind="ExternalInput")
    ident_d = nc.dram_tensor("ident", [P, P], F32, kind="ExternalInput")
    onesr_d = nc.dram_tensor("ones_row", [1, P], F32, kind="ExternalInput")
    valid_d = nc.dram_tensor("valid2", [P, 2], F32, kind="ExternalInput")
    gb_d = nc.dram_tensor("gammabeta", [1, 2 * P], F32, kind="ExternalInput")
    w1_d = nc.dram_tensor("w1s", [n_layers, P, P], F32, kind="ExternalInput")
    w2_d = nc.dram_tensor("w2s", [n_layers, P, P], F32, kind="ExternalInput")
    b1t_d = nc.dram_tensor("b1T", [P, n_layers], F32, kind="ExternalInput")
    b2r_d = nc.dram_tensor("b2rows", [1, n_layers * P], F32, kind="ExternalInput")
    wp_d = nc.dram_tensor("wp", [P, P], F32, kind="ExternalInput")
    bpt_d = nc.dram_tensor("bpT", [P, 1], F32, kind="ExternalInput")
    pa_d = nc.dram_tensor("paT", [P, 1], F32, kind="ExternalInput")

    zn_out = nc.dram_tensor("zn_out", [NPC, P], F32, kind="ExternalOutput")
    dbg_out = (nc.dram_tensor("dbg_out", [P, NPC], F32, kind="ExternalOutput")
               if debug_dump else None)
    dbg2_out = (nc.dram_tensor("dbg2_out", [NPC, P], F32, kind="ExternalOutput")
                if debug_dump else None)
    dbg3_out = (nc.dram_tensor("dbg3_out", [P, 2 * P], F32, kind="ExternalOutput")
                if debug_dump else None)
    dbg4_out = (nc.dram_tensor("dbg4_out", [1, 2 * P], F32, kind="ExternalOutput")
                if debug_dump else None)
    pt_out = nc.dram_tensor("pT_out", [P, NPC], F32, kind="ExternalOutput")

    rg = [list(range(n_cores))]

    with tile.TileContext(nc) as tc:
        with (
            tc.tile_pool(name="const", bufs=1) as cpool,
            tc.tile_pool(name="meta", bufs=1) as mpool,
            tc.tile_pool(name="wl", bufs=2) as wlpool,
            tc.tile_pool(name="zg", bufs=8) as zgpool,
            tc.tile_pool(name="mask", bufs=6) as maskpool,
            tc.tile_pool(name="mlp", bufs=3) as mlppool,
            tc.tile_pool(name="z3keep", bufs=NT + 1) as z3pool,
            tc.tile_pool(name="small", bufs=1) as spool,
            tc.tile_pool(name="aggp", bufs=2, space="PSUM") as aggp,
            tc.tile_pool(name="mmp", bufs=2, space="PSUM") as mmp,
            tc.tile_pool(name="bcp", bufs=1, space="PSUM") as bcp,
            tc.tile_pool(name="statp", bufs=1, space="PSUM") as statp,
            tc.tile_pool(name="dram", bufs=1, space="DRAM") as dpool,
        ):
            # ---- constants / metadata loads ----
            iota_sb = cpool.tile([P, sc.NCHP_MAX * P], F32)
            nc.sync.dma_start(iota_sb[:], iota_d[:])
            ident_sb = cpool.tile([P, P], F32)
            nc.sync.dma_start(ident_sb[:], ident_d[:])
            onesr_sb = cpool.tile([1, P], F32)
            nc.sync.dma_start(onesr_sb[:], onesr_d[:])
            valid_sb = cpool.tile([P, 2], F32)
            nc.sync.dma_start(valid_sb[:], valid_d[:])
            gb_sb = cpool.tile([1, 2 * P], F32)
            nc.sync.dma_start(gb_sb[:], gb_d[:])
            w1_sb = cpool.tile([P, n_layers * P], F32)
            w2_sb = cpool.tile([P, n_layers * P], F32)
            for l in range(layers_run):
                nc.sync.dma_start(w1_sb[:, l * P:(l + 1) * P], w1_d[l, :, :])
                nc.sync.dma_start(w2_sb[:, l * P:(l + 1) * P], w2_d[l, :, :])
            b1t_sb = cpool.tile([P, n_layers], F32)
            nc.sync.dma_start(b1t_sb[:], b1t_d[:])
            b2r_sb = cpool.tile([1, n_layers * P], F32)
            nc.sync.dma_start(b2r_sb[:], b2r_d[:])
            wp_sb = cpool.tile([P, P], F32)
            nc.sync.dma_start(wp_sb[:], wp_d[:])
            bpt_sb = cpool.tile([P, 1], F32)
            nc.sync.dma_start(bpt_sb[:], bpt_d[:])
            pa_sb = cpool.tile([P, 1], F32)
            nc.sync.dma_start(pa_sb[:], pa_d[:])

            idx_sb = mpool.tile([128, sc.IDXCOLS], I16)
            nc.sync.dma_start(idx_sb[:], idx16[:])
            dstl_sb = mpool.tile([P, sc.NCHTOT], F32)
            nc.sync.dma_start(dstl_sb[:], dstl_d[:])

            # z tables + AG buffers (layers 0..n_layers-2 produce a new table)
            zshard = [dpool.tile([NPC, P], F32, name=f"zshard{i}")
                      for i in range(n_layers - 1)]
            zbuf = [dpool.tile([N, P], F32, name=f"zbuf{i}")
                    for i in range(n_layers - 1)]

            stats_ps = statp.tile([1, 2 * P], F32, space="PSUM")

            z3_tiles = []
            qctr = [0]

            for l in range(layers_run):
                w_sb = wlpool.tile([P, sc.NCHTOT], F32)
                nc.sync.dma_start(
                    w_sb[:], wts_d[:, l * sc.NCHTOT:(l + 1) * sc.NCHTOT])

                table = xfull if l == 0 else zbuf[l - 1]
                tabA = table[0:min(N, A_LIM), :]
                tabB = table[B_OFF:N, :] if has_b else None

                for gi, tiles in enumerate(sc.groups):
                    if not do_gather:
                        if gi == 0:
                            tmp7 = mlppool.tile([P, P], F32, tag="h", name="tmp7")
                            nc.vector.tensor_copy(out=tmp7[:], in_=iota_sb[:, 0:P])
                            nc.sync.dma_start(zn_out[0:P, :], tmp7[:])
                            nc.sync.dma_start(pt_out[:, 0:P], tmp7[:])
                        continue
                    agg_ps = aggp.tile([P, sc.GROUP * P], F32, space="PSUM")

                    # Per-call gather -> per-segment mask build -> matmuls.
                    # PSUM `start=True` resets the whole 2KB bank zero-region,
                    # so only the very first matmul touching this bank may
                    # carry start=True; every other accumulation chain relies
                    # on first-touch-zero and is ordered after the opener.
                    bank_opener = None
                    first_chunk = {t: True for t in tiles}
                    chunks_total = {t: int(sc.nch[t, 0] + sc.nch[t, 1])
                                    for t in tiles}
                    chunks_done = {t: 0 for t in tiles}
                    for pt, entry in sc.calls_by_group[gi]:
                        nch_call = sum(cc for _, _, cc in entry)
                        n_idx = nch_call * P
                        zg = zgpool.tile([P, nch_call, P], F32, tag="zg",
                                         name="zg")
                        c0 = entry[0][1] * P // 16
                        nc.gpsimd.dma_gather(
                            zg[:], tabA if pt == 0 else tabB,
                            idx_sb[:, c0:c0 + n_idx // 16],
                            n_idx, n_idx, P,
                            single_packet=False, queue_num=qctr[0] % 4)
                        qctr[0] += 1
                        zoff = 0
                        for t, coff, nseg in entry:
                            if not do_mask:
                                mk0 = maskpool.tile([P, 1, P], F32,
                                                    tag="cns", name="mk0")
                                nc.vector.tensor_copy(
                                    out=mk0[:], in_=zg[:, zoff, :])
                                zoff += nseg
                                continue
                            mk = maskpool.tile([P, nseg, P], F32, tag="mask",
                                               name="mk")
                            iota3 = iota_sb[:, :nseg * P].rearrange(
                                "p (c d) -> p c d", d=P)
                            nc.vector.tensor_tensor(
                                out=mk[:], in0=iota3,
                                in1=dstl_sb[:, coff:coff + nseg].to_broadcast(
                                    [P, nseg, P]),
                                op=ALU.is_equal)
                            nc.vector.tensor_tensor(
                                out=mk[:], in0=mk[:],
                                in1=w_sb[:, coff:coff + nseg].to_broadcast(
                                    [P, nseg, P]),
                                op=ALU.mult)
                            tslot = t - tiles[0]
                            if not do_agg:
                                zoff += nseg
                                continue
                            for j in range(nseg):
                                mm = nc.tensor.matmul(
                                    out=agg_ps[:, tslot * P:(tslot + 1) * P],
                                    lhsT=zg[:, zoff + j, :],
                                    rhs=mk[:, j, :],
                                    start=(bank_opener is None),
                                    stop=(chunks_done[t] + j + 1
                                          == chunks_total[t]),
                                    skip_group_check=True)
                                if bank_opener is None:
                                    bank_opener = mm.ins
                                elif first_chunk[t]:
                                    tile.add_dep_helper(
                                        mm.ins, bank_opener,
                                        reason="psum bank first-touch order")
                                first_chunk[t] = False
                            chunks_done[t] += nseg
                            zoff += nseg

                    # MLP per tile
                    for t in tiles:
                        if not do_agg:
                            continue
                        if not do_mlp:
                            h_dbg = mlppool.tile([P, P], F32, tag="h", name="h_dbg")
                            nc.scalar.copy(
                                out=h_dbg[:],
                                in_=agg_ps[:, (t - tiles[0]) * P:(t - tiles[0] + 1) * P])
                            vr0 = last_rows if t == NT - 1 else P
                            nc.sync.dma_start(
                                zn_out[t * P:t * P + vr0, :], h_dbg[:vr0, :])
                            nc.sync.dma_start(
                                pt_out[:, t * P:t * P + vr0], h_dbg[:, :vr0])
                            continue
                        tslot = t - tiles[0]
                        vr = last_rows if t == NT - 1 else P
                        h_sb = mlppool.tile([P, P], F32, tag="h")
                        nc.scalar.copy(
                            out=h_sb[:], in_=agg_ps[:, tslot * P:(tslot + 1) * P])
                        if debug_dump and l == 0:
                            nc.sync.dma_start(
                                dbg_out[:, t * P:t * P + (last_rows if t == NT - 1 else P)],
                                h_sb[:, :last_rows if t == NT - 1 else P])
                        ps1 = mmp.tile([P, P], F32, space="PSUM", tag="ps1")
                        nc.tensor.matmul(out=ps1[:], lhsT=w1_sb[:, l * P:(l + 1) * P],
                                         rhs=h_sb[:], start=True, stop=True,
                                         skip_group_check=True)
                        h1_sb = mlppool.tile([P, P], F32, tag="h1")
                        nc.scalar.activation(out=h1_sb[:], in_=ps1[:], func=AF.Relu,
                                             bias=b1t_sb[:, l:l + 1], scale=1.0)
                        ps2 = mmp.tile([P, P], F32, space="PSUM", tag="ps2")
                        nc.tensor.matmul(out=ps2[:], lhsT=h1_sb[:],
                                         rhs=w2_sb[:, l * P:(l + 1) * P],
                                         start=True, stop=False,
                                         skip_group_check=True)
                        nc.tensor.matmul(out=ps2[:], lhsT=onesr_sb[:],
                                         rhs=b2r_sb[0:1, l * P:(l + 1) * P],
                                         start=False, stop=True,
                                         skip_group_check=True)
                        if l == n_layers - 1:
                            z3 = z3pool.tile([P, P], F32, name=f"z3k{t}",
                                             tag="z3k")
                        else:
                            z3 = mlppool.tile([P, P], F32, tag="z3", name="z3")
                        nc.scalar.activation(out=z3[:], in_=ps2[:], func=AF.Relu)
                        if debug_dump and l == n_layers - 1:
                            nc.sync.dma_start(
                                dbg2_out[t * P:t * P + vr, :], z3[:vr, :])
                        if l < n_layers - 1 or not do_tail:
                            if l < n_layers - 1:
                                nc.sync.dma_start(
                                    zshard[l][t * P:t * P + vr, :], z3[:vr, :])
                            if not do_tail and l == layers_run - 1:
                                nc.sync.dma_start(
                                    zn_out[t * P:t * P + vr, :], z3[:vr, :])
                                nc.sync.dma_start(
                                    pt_out[:, t * P:t * P + vr], z3[:, :vr])
                        else:
                            z3_tiles.append(z3)
                            vi = 1 if t == NT - 1 else 0
                            # single start=True for the shared stats bank
                            # (S at t==0); Q relies on first-touch-zero.
                            mm_s = nc.tensor.matmul(
                                out=stats_ps[:, 0:P],
                                lhsT=valid_sb[:, vi:vi + 1], rhs=z3[:],
                                start=(t == 0), stop=(t == NT - 1),
                                skip_group_check=True)
                            if t == 0:
                                stats_opener = mm_s.ins
                            sq = mlppool.tile([P, P], F32, tag="sq")
                            nc.scalar.activation(out=sq[:], in_=z3[:],
                                                 func=AF.Square)
                            mm_q = nc.tensor.matmul(
                                out=stats_ps[:, P:2 * P],
                                lhsT=valid_sb[:, vi:vi + 1], rhs=sq[:],
                                start=False, stop=(t == NT - 1),
                                skip_group_check=True)
                            if t == 0:
                                tile.add_dep_helper(
                                    mm_q.ins, stats_opener,
                                    reason="stats psum bank first-touch order")

                if l < n_layers - 1 and do_coll:
                    nc.gpsimd.collective_compute(
                        "AllGather", ALU.bypass,
                        ins=[zshard[l].opt()], outs=[zbuf[l].opt()],
                        replica_groups=rg)

            # ---- BatchNorm stats across cores ----
            if do_tail:
                stats_sb = spool.tile([1, 2 * P], F32)
                nc.vector.tensor_copy(out=stats_sb[:], in_=stats_ps[:])
                ar_in = dpool.tile([1, 2 * P], F32)
                ar_out = dpool.tile([1, 2 * P], F32)
                nc.sync.dma_start(ar_in[:], stats_sb[:])
                nc.gpsimd.collective_compute(
                    "AllReduce", ALU.add, ins=[ar_in.opt()], outs=[ar_out.opt()],
                    replica_groups=rg)
                gstats = spool.tile([1, 2 * P], F32)
                nc.sync.dma_start(gstats[:], ar_out[:])

                mean = spool.tile([1, P], F32)
                nc.vector.tensor_scalar(out=mean[:], in0=gstats[:, 0:P],
                                        scalar1=1.0 / N, scalar2=None, op0=ALU.mult)
                msq = spool.tile([1, P], F32)
                nc.vector.tensor_scalar(out=msq[:], in0=gstats[:, P:2 * P],
                                        scalar1=1.0 / N, scalar2=None, op0=ALU.mult)
                var = spool.tile([1, P], F32)
                nc.vector.tensor_tensor(out=var[:], in0=mean[:], in1=mean[:],
                                        op=ALU.mult)
                nc.vector.tensor_tensor(out=var[:], in0=msq[:], in1=var[:],
                                        op=ALU.subtract)
                nc.vector.tensor_scalar(out=var[:], in0=var[:], scalar1=BN_EPS,
                                        scalar2=None, op0=ALU.add)
                sd = spool.tile([1, P], F32)
                nc.scalar.activation(out=sd[:], in_=var[:], func=AF.Sqrt)
                rstd = spool.tile([1, P], F32)
                nc.vector.reciprocal(out=rstd[:], in_=sd[:])
                s_row = spool.tile([1, P], F32)
                nc.vector.tensor_tensor(out=s_row[:], in0=gb_sb[0:1, 0:P], in1=rstd[:],
                                        op=ALU.mult)
                t_row = spool.tile([1, P], F32)
                nc.vector.tensor_tensor(out=t_row[:], in0=mean[:], in1=s_row[:],
                                        op=ALU.mult)
                nc.vector.tensor_tensor(out=t_row[:], in0=gb_sb[0:1, P:2 * P], in1=t_row[:],
                                        op=ALU.subtract)

                # broadcast s,t to [P, P] via K=1 matmul
                ps_bc = bcp.tile([P, 2 * P], F32, space="PSUM", tag="bc")
                mm_bs = nc.tensor.matmul(out=ps_bc[:, 0:P], lhsT=onesr_sb[:],
                                         rhs=s_row[:], start=True, stop=True,
                                         skip_group_check=True)
                mm_bt = nc.tensor.matmul(out=ps_bc[:, P:2 * P], lhsT=onesr_sb[:],
                                         rhs=t_row[:], start=False, stop=True,
                                         skip_group_check=True)
                tile.add_dep_helper(mm_bt.ins, mm_bs.ins,
                                    reason="bc psum bank first-touch order")
                s_bc = spool.tile([P, P], F32)
                nc.vector.tensor_copy(out=s_bc[:], in_=ps_bc[:, 0:P])
                t_bc = spool.tile([P, P], F32)
                nc.vector.tensor_copy(out=t_bc[:], in_=ps_bc[:, P:2 * P])
                if debug_dump:
                    nc.sync.dma_start(dbg3_out[:, 0:P], s_bc[:])
                    nc.sync.dma_start(dbg3_out[:, P:2 * P], t_bc[:])
                    nc.sync.dma_start(dbg4_out[:], gstats[:])

                # ---- normalize + projection + PReLU ----
                for t in range(NT):
                    vr = last_rows if t == NT - 1 else P
                    z3 = z3_tiles[t]
                    zn_t = mlppool.tile([P, P], F32, tag="zn")
                    nc.vector.tensor_tensor(out=zn_t[:], in0=z3[:], in1=s_bc[:],
                                            op=ALU.mult)
                    nc.vector.tensor_tensor(out=zn_t[:], in0=zn_t[:], in1=t_bc[:],
                                            op=ALU.add)
                    nc.sync.dma_start(zn_out[t * P:t * P + vr, :], zn_t[:vr, :])

                    ps_tr = mmp.tile([P, P], F32, space="PSUM", tag="ps1")
                    nc.tensor.transpose(out=ps_tr[:], in_=zn_t[:],
                                        identity=ident_sb[:])
                    znT = mlppool.tile([P, P], F32, tag="znT")
                    nc.vector.tensor_copy(out=znT[:], in_=ps_tr[:])
                    ps_p = mmp.tile([P, P], F32, space="PSUM", tag="ps2")
                    nc.tensor.matmul(out=ps_p[:], lhsT=wp_sb[:], rhs=znT[:],
                                     start=True, stop=True, skip_group_check=True)
                    x_sb = mlppool.tile([P, P], F32, tag="x")
                    nc.scalar.activation(out=x_sb[:], in_=ps_p[:], func=AF.Identity,
                                         bias=bpt_sb[:], scale=1.0)
                    neg = mlppool.tile([P, P], F32, tag="neg")
                    nc.vector.tensor_scalar(out=neg[:], in0=x_sb[:], scalar1=0.0,
                                            scalar2=pa_sb[:], op0=ALU.min,
                                            op1=ALU.mult)
                    pos = mlppool.tile([P, P], F32, tag="pos")
                    nc.scalar.activation(out=pos[:], in_=x_sb[:], func=AF.Relu)
                    p_t = mlppool.tile([P, P], F32, tag="pt")
                    nc.vector.tensor_tensor(out=p_t[:], in0=pos[:], in1=neg[:],
                                            op=ALU.add)
                    nc.sync.dma_start(pt_out[:, t * P:t * P + vr], p_t[:, :vr])

    nc.compile()
    return nc


# ----------------------------------------------------------------------------
# Entry point
# ----------------------------------------------------------------------------

_CACHE = {}


def _get_compiled(edge_index, edge_weight, eps, n_cores, N):
    key = (N, n_cores, hash(edge_index.tobytes()), hash(edge_weight.tobytes()),
           hash(np.asarray(eps).tobytes()))
    if key in _CACHE:
        return _CACHE[key]
    ope = 1.0 + np.asarray(eps, np.float64)
    sc = _preprocess(edge_index, edge_weight, ope, N, n_cores)
    nc = _build(sc, n_layers=len(ope))
    _CACHE[key] = (sc, nc)
    return sc, nc


def kernel(x, edge_weight, W1s, b1s, W2s, b2s, eps, gamma, beta, Wp, bp,
           prelu_a, edge_index, n_cores=8):
    x = np.ascontiguousarray(np.asarray(x, np.float32))
    N, D = x.shape
    assert D == P
    sc, nc = _get_compiled(np.asarray(edge_index), np.asarray(edge_weight),
                           np.asarray(eps), n_cores, N)
    n_layers = len(np.asarray(eps))

    iota_rep = np.tile(np.arange(P, dtype=np.float32), (P, sc.NCHP_MAX))
    ident = np.eye(P, dtype=np.float32)
    ones_row = np.ones((1, P), np.float32)
    valid2 = np.ones((P, 2), np.float32)
    last_rows = sc.NPC - (sc.NT - 1) * P
    valid2[last_rows:, 1] = 0.0
    gammabeta = np.concatenate([np.asarray(gamma, np.float32),
                                np.asarray(beta, np.float32)]).reshape(1, -1)
    common = {
        "xfull": x,
        "iota_rep": iota_rep,
        "ident": ident,
        "ones_row": ones_row,
        "valid2": valid2,
        "gammabeta": gammabeta,
        "w1s": np.ascontiguousarray(np.asarray(W1s, np.float32)),
        "w2s": np.ascontiguousarray(np.asarray(W2s, np.float32)),
        "b1T": np.ascontiguousarray(np.asarray(b1s, np.float32).T),
        "b2rows": np.ascontiguousarray(np.asarray(b2s, np.float32).reshape(1, -1)),
        "wp": np.ascontiguousarray(np.asarray(Wp, np.float32)),
        "bpT": np.asarray(bp, np.float32).reshape(P, 1),
        "paT": np.full((P, 1), np.float32(np.asarray(prelu_a))),
    }
    in_maps = []
    for c in range(n_cores):
        m = dict(common)
        m["idx16"] = sc.idx16[c]
        m["dstl"] = sc.dstl[c]
        m["wts"] = sc.wts[c]
        in_maps.append(m)

    res = run_bass_kernel_spmd(nc, in_maps, core_ids=list(range(n_cores)))
    zn = np.concatenate([res.results[c]["zn_out"] for c in range(n_cores)],
                        axis=0)
    p = np.concatenate([res.results[c]["pT_out"].T for c in range(n_cores)],
                       axis=0)
    return zn, p

